# revision 1
# baseline (speedup 1.0000x reference)
"""Trainium2 Bass kernel for stacked ConvLSTM2D (4 layers, Keras semantics).

Scheme: space-to-depth s=2 block layout; each conv is a sum of block-tap
matmuls with K padded to 128 via shift-baked replica buffers (all matmuls
K=128, M=128, N=CR*Wb, bf16 in / f32 PSUM).

8-core SPMD: batch b on core pair (2b, 2b+1), split over image rows.
Odd cores solve a vertically-FLIPPED half (flipped x + flipped weight slabs
prepped on host) so the program is rank-symmetric: every core owns local
block rows 0..49 (L1: 0..59), its local top is a true image boundary, and
its exchange edge is its local bottom.
 - L1 runs 60 rows with no exchange: validity of the extra 10 overlap rows
   decays 1 row/step; after 10 steps rows <=50 are still valid, exactly
   what L2 needs.
 - L2..L4 exchange a 4-block-row halo every step: each core sends its rows
   46..49 (siy-swapped via two partition-block DMAs; channel order is
   siy-major so the swap is two contiguous partition ranges), pair
   AllReduce(add) in DRAM, then halo = sum - own (exact peer recovery),
   written row-reversed into frame rows 54..57.
 - hrep rebuild is split into main/halo DMAs so only the boundary chunk
   (ci=9) depends on the exchange; chunks 0..8 of the next timestep keep
   the PE busy while the collective flies.
Gates: M-order (i, f, g, o), gate blocks of 4*Fpad partitions, in-block
order (soy, f, sox). State c stays f32 in SBUF; h is bf16.
"""
import math
import os
from contextlib import ExitStack

import numpy as np
import ml_dtypes

import concourse.bacc as bacc
import concourse.bass as bass
import concourse.mybir as mybir
from concourse.tile import TileContext
from concourse.bass_utils import run_bass_kernel_spmd

BF16 = mybir.dt.bfloat16
F32 = mybir.dt.float32
AF = mybir.ActivationFunctionType
ALU = mybir.AluOpType

S = 2
PB = 4
# (cin_raw, F, k, tap_radius R)
LAYERS = [(1, 8, 3, 1), (8, 16, 5, 1), (16, 16, 9, 2), (16, 5, 12, 3)]
FPAD = [8, 16, 16, 8]
CINPAD = [1, 8, 16, 16]
N_CORES = 8
RG = [[0, 1], [2, 3], [4, 5], [6, 7]]

Wb = 100
WbP = Wb + 2 * PB          # 108
HBO1 = 60                  # L1 owned block rows per core
HP1 = HBO1 + 2 * PB        # 68
FLAT1 = HP1 * WbP          # 7344
HBO = 50                   # L2..L4 owned block rows per core
HP2 = HBO + 2 * PB         # 58
FLAT2 = HP2 * WbP          # 6264
HS = (PB + HBO) * WbP      # 5832, halo start (frame row 54)
CR = 5
T = 10


# ---------------------------------------------------------------- host prep --

def same_pad_lo(k):
    return (k - 1) // 2


def s2d_np(img):
    """[H, W, C] -> [4C, Hb, Wb], channel = siy*2C + c*2 + six (siy-major)."""
    H, W, C = img.shape
    Hb, Wbl = H // S, W // S
    t = img.reshape(Hb, S, Wbl, S, C)          # hb, siy, wb, six, c
    return t.transpose(1, 4, 3, 0, 2).reshape(4 * C, Hb, Wbl)


def un_s2d_np(blk, C, Cp, H, W):
    """blk [4Cp(siy,f,six), Hb, Wb] -> [H, W, C] (first C of Cp channels)."""
    Hb, Wbl = H // S, W // S
    b = blk.reshape(S, Cp, S, Hb, Wbl)[:, :C]  # siy, c, six, hb, wb
    return b.transpose(3, 0, 4, 2, 1).reshape(H, W, C)


def remap_kernel(Wk, cin_pad, F, Fp):
    k = Wk.shape[0]
    cin = Wk.shape[2]
    out = np.zeros((k, k, cin_pad, 4 * Fp), np.float32)
    for g in range(4):
        out[:, :, :cin, g * Fp:g * Fp + F] = Wk[:, :, :, g * F:(g + 1) * F]
    return out


def block_weights(Wk, pt, pl, R):
    """Wk [k,k,cinp,4Fp] -> dict[(by,bx)] of [4cinp, 16Fp] with
    row = siy*2cinp + ci*2 + six, col = g*4Fp + soy*2Fp + f*2 + sox."""
    k, _, cinp, coutp = Wk.shape
    Fp = coutp // 4
    out = {}
    for by in range(-R, R + 1):
        for bx in range(-R, R + 1):
            M = np.zeros((4 * cinp, 4 * coutp), np.float32)
            Mv = M.reshape(4 * cinp, 4, S, Fp, S)
            for siy in range(S):
                for six in range(S):
                    for soy in range(S):
                        for sox in range(S):
                            dy = S * by + siy - soy + pt
                            dx = S * bx + six - sox + pl
                            if 0 <= dy < k and 0 <= dx < k:
                                rows = slice(siy * 2 * cinp + six,
                                             siy * 2 * cinp + 2 * cinp, 2)
                                Mv[rows, :, soy, :, sox] = \
                                    Wk[dy, dx].reshape(cinp, 4, Fp)
            out[(by, bx)] = M
    return out


def conv_groups(li, conv):
    """Returns (K, reps, groups) where groups = [(bdy, bdx0)]."""
    R = LAYERS[li][3]
    K = 4 * (CINPAD[li] if conv == 'x' else FPAD[li])
    reps = 128 // K
    ngroups_x = math.ceil((2 * R + 1) / reps)
    groups = [(bdy, -R + m * reps)
              for bdy in range(-R, R + 1) for m in range(ngroups_x)]
    return K, reps, groups


def build_slabs(Wblk, li, conv):
    """-> np [nz*G, 128, 128] f32 (caller casts to bf16). Order: [zh][group]."""
    R = LAYERS[li][3]
    K, reps, groups = conv_groups(li, conv)
    nz = 2 if FPAD[li] == 16 else 1
    slabs = []
    for zh in range(nz):
        for (bdy, bdx0) in groups:
            slab = np.zeros((128, 128), np.float32)
            for j in range(reps):
                bdx = bdx0 + j
                if bdx > R:
                    continue
                slab[j * K:(j + 1) * K, :] = Wblk[(bdy, bdx)][:, zh * 128:(zh + 1) * 128]
            slabs.append(slab)
    return np.stack(slabs)


def prep_core_inputs(inputs, b, half):
    """Build the input map for core 2b+half (half 1 = vertically flipped)."""
    m = {}
    for li, (cin, F, k, R) in enumerate(LAYERS):
        pl = same_pad_lo(k)
        pt = pl if half == 0 else k - 1 - pl
        Fp = FPAD[li]
        Wx = np.asarray(inputs[f'Wx{li+1}'], np.float32)
        Wh = np.asarray(inputs[f'Wh{li+1}'], np.float32)
        if half == 1:
            Wx, Wh = Wx[::-1].copy(), Wh[::-1].copy()
        Wxb = block_weights(remap_kernel(Wx, CINPAD[li], F, Fp), pt, pl, R)
        Whb = block_weights(remap_kernel(Wh, Fp, F, Fp), pt, pl, R)
        braw = np.asarray(inputs[f'b{li+1}'], np.float32)
        bex = np.zeros(16 * Fp, np.float32)
        bexv = bex.reshape(4, S, Fp, S)
        for g in range(4):
            for f in range(F):
                bexv[g, :, f, :] = braw[g * F + f]
        NFp = 4 * Fp
        if Fp == 16:
            b1 = 0.2 * bex[:2 * NFp] + 0.5                             # (i,f) hsig'
            b2 = np.concatenate([bex[2 * NFp:3 * NFp],                 # g raw
                                 0.2 * bex[3 * NFp:] + 0.5])           # o hsig'
            m[f'bias{li+1}a'] = b1.reshape(128, 1)
            m[f'bias{li+1}b'] = b2.reshape(128, 1)
        else:
            b1 = np.concatenate([0.2 * bex[:2 * NFp] + 0.5,            # i,f
                                 bex[2 * NFp:3 * NFp],                 # g raw
                                 0.2 * bex[3 * NFp:] + 0.5])           # o
            m[f'bias{li+1}a'] = b1.reshape(128, 1)
        if li == 0:
            # L1 x: single K=36 im2col slab (tap-major rows), 1 group
            slab = np.zeros((128, 128), np.float32)
            for t_i, (bdy, bdx) in enumerate(
                    (by, bx) for by in range(-R, R + 1) for bx in range(-R, R + 1)):
                slab[t_i * 4:(t_i + 1) * 4, :] = Wxb[(bdy, bdx)]
            m['wx1'] = slab[None].astype(ml_dtypes.bfloat16)
        else:
            m[f'wx{li+1}'] = build_slabs(Wxb, li, 'x').astype(ml_dtypes.bfloat16)
        m[f'wh{li+1}'] = build_slabs(Whb, li, 'h').astype(ml_dtypes.bfloat16)

    # L1 x im2col: local 120 pixel rows (60 block rows), frame 68 block rows
    x = np.asarray(inputs['x'], np.float32)[b]          # [T, 200, 200, 1]
    if half == 1:
        x = x[:, ::-1]
    xc = np.zeros((T, 36, FLAT1 + 8), np.float32)
    for t in range(T):
        xp = np.zeros((4, HP1, WbP), np.float32)
        xp[:, PB:PB + HBO1, PB:PB + Wb] = s2d_np(x[t, 0:2 * HBO1])
        flat = xp.reshape(4, FLAT1)
        for t_i, (bdy, bdx) in enumerate(
                (by, bx) for by in (-1, 0, 1) for bx in (-1, 0, 1)):
            sh = bdy * WbP + bdx
            for c in range(4):
                if sh >= 0:
                    xc[t, t_i * 4 + c, :FLAT1 - sh] = flat[c, sh:]
                else:
                    xc[t, t_i * 4 + c, -sh:FLAT1] = flat[c, :FLAT1 + sh]
    m['xcol'] = xc.astype(ml_dtypes.bfloat16)
    return m


# ------------------------------------------------------------- kernel build --

def build_kernel(static_unroll=True):
    nc = bacc.Bacc("TRN2", target_bir_lowering=False, debug=False,
                   num_devices=N_CORES)

    xcol = nc.dram_tensor('xcol', [T, 36, FLAT1 + 8], BF16, kind="ExternalInput")
    wts, biases = {}, {}
    for li in range(4):
        nz = 2 if FPAD[li] == 16 else 1
        Kx, repx, gx = conv_groups(li, 'x')
        Kh, reph, gh = conv_groups(li, 'h')
        Gx = 1 if li == 0 else nz * len(gx)
        Gh = nz * len(gh)
        wts[(li, 'x')] = nc.dram_tensor(f'wx{li+1}', [Gx, 128, 128], BF16, kind="ExternalInput")
        wts[(li, 'h')] = nc.dram_tensor(f'wh{li+1}', [Gh, 128, 128], BF16, kind="ExternalInput")
        biases[(li, 'a')] = nc.dram_tensor(f'bias{li+1}a', [128, 1], F32, kind="ExternalInput")
        if nz == 2:
            biases[(li, 'b')] = nc.dram_tensor(f'bias{li+1}b', [128, 1], F32, kind="ExternalInput")
    hseqs = [nc.dram_tensor(f'hseq{li+1}', [T + 1, 4 * FPAD[li], FLAT2 + 8], BF16,
                            kind="Internal")
             for li in range(3)]
    out = nc.dram_tensor('out', [T, 32, HBO * Wb], F32, kind="ExternalOutput")

    with TileContext(nc) as tc, ExitStack() as top:
        gp = top.enter_context(tc.tile_pool(name="glob", bufs=1))
        xrA = gp.tile([128, FLAT1], BF16, tag="xrA")
        xrB = gp.tile([128, FLAT1], BF16, tag="xrB")
        hrA = gp.tile([128, FLAT1], BF16, tag="hrA")
        hrB = gp.tile([128, FLAT1], BF16, tag="hrB")
        nc.vector.memset(xrA[:, :], 0.0)
        nc.vector.memset(xrB[:, :], 0.0)

        # all layers' weights + biases loaded up-front (kills layer-boundary
        # PE stalls waiting on slab DMAs)
        wxts, whts, bias_t = {}, {}, {}
        for li in range(4):
            Gx = wts[(li, 'x')].shape[0]
            Gh = wts[(li, 'h')].shape[0]
            wxts[li] = gp.tile([128, Gx * 128], BF16, tag=f"wx{li}", name=f"wxt{li}")
            whts[li] = gp.tile([128, Gh * 128], BF16, tag=f"wh{li}", name=f"wht{li}")
            nc.sync.dma_start(wxts[li][:, :].rearrange("p (g c) -> p g c", c=128),
                              wts[(li, 'x')].ap().rearrange("g p c -> p g c"))
            nc.sync.dma_start(whts[li][:, :].rearrange("p (g c) -> p g c", c=128),
                              wts[(li, 'h')].ap().rearrange("g p c -> p g c"))
            bias_t[(li, 'a')] = gp.tile([128, 1], F32, tag=f"ba{li}", name=f"bat{li}")
            nc.sync.dma_start(bias_t[(li, 'a')][:, :], biases[(li, 'a')].ap())
            if FPAD[li] == 16:
                bias_t[(li, 'b')] = gp.tile([128, 1], F32, tag=f"bb{li}", name=f"bbt{li}")
                nc.sync.dma_start(bias_t[(li, 'b')][:, :], biases[(li, 'b')].ap())

        for li in range(4):
            cin, F, k, R = LAYERS[li]
            Fp = FPAD[li]
            NFp = 4 * Fp
            nz = 2 if Fp == 16 else 1
            Kx, repx, gx = conv_groups(li, 'x')
            Kh, reph, gh = conv_groups(li, 'h')
            if li == 0:
                gx_list = [[(0, 0, 0)]]  # shifts baked into xcol data
            else:
                gx_list = [[(zh * len(gx) + i, bdy, bdx0)
                            for i, (bdy, bdx0) in enumerate(gx)] for zh in range(nz)]
            gh_list = [[(zh * len(gh) + i, bdy, bdx0)
                        for i, (bdy, bdx0) in enumerate(gh)] for zh in range(nz)]

            FLATl = FLAT1 if li == 0 else FLAT2
            HBOl = HBO1 if li == 0 else HBO
            NCHl = HBOl // CR

            NG = 6 if li == 0 else 5          # chunks per vector group
            NGR = NG * CR                     # rows per group
            XR = {0: 0, 1: 2, 2: 3, 3: 3}[li]  # exchanged halo rows needed

            with ExitStack() as ls:
                lp = ls.enter_context(tc.tile_pool(name=f"l{li}", bufs=1))
                pp = ls.enter_context(tc.tile_pool(name=f"ps{li}", bufs=4, space="PSUM"))
                tp = ls.enter_context(tc.tile_pool(name=f"tmp{li}", bufs=2))
                if li > 0:
                    dp = ls.enter_context(tc.tile_pool(name=f"xch{li}", bufs=2,
                                                       space="DRAM"))

                wxt, wht = wxts[li], whts[li]
                bia = bias_t[(li, 'a')]
                if nz == 2:
                    bib = bias_t[(li, 'b')]
                # gate staging (scalar engine drains PSUM into these)
                PA = 2 * NFp                          # i,f partitions
                PG = NFp                              # g (and o) partitions
                As = lp.tile([128, HBOl, Wb], BF16, tag="As")
                Gs = lp.tile([128, HBOl, Wb], BF16, tag="Gs")
                Os = lp.tile([128, HBOl, Wb], BF16, tag="Os")
                TCs = lp.tile([128, HBOl, Wb], BF16, tag="TCs")

                H = lp.tile([NFp, FLATl + 8], BF16, tag="H")
                C = lp.tile([128, HBOl, Wb], F32, tag="C")
                nc.vector.memset(H[:, :], 0.0)
                nc.vector.memset(C[:, :, :], 0.0)
                if li == 3:
                    OS = lp.tile([32, HBO, Wb], F32, tag="OS")
                if li < 3:
                    nc.sync.dma_start(hseqs[li].ap()[0, :, :], H[:, 0:FLAT2 + 8])
                if li > 0:
                    SB = lp.tile([128, XR, WbP], BF16, tag="SB")
                    Rt = lp.tile([128, XR, WbP], BF16, tag="Rt")

                H3 = H[:, 0:FLATl].rearrange("p (h w) -> p h w", w=WbP)

                def build_xr(t, buf):
                    if li == 0:
                        nc.sync.dma_start(buf[0:36, 0:FLAT1],
                                          xcol.ap()[bass.ds(t, 1), :, 0:FLAT1])
                    else:
                        src = hseqs[li - 1].ap()
                        for j in range(repx):
                            nc.sync.dma_start(
                                buf[j * Kx:(j + 1) * Kx, 0:FLAT2],
                                src[bass.ds(t + 1, 1), 0:Kx, j:j + FLAT2])

                def band_range(b):
                    lo = 0 if b == 0 else (PB + b * CR) * WbP
                    if b == NCHl - 1:
                        hi = FLAT1 if li == 0 else HS - 8
                    else:
                        hi = (PB + (b + 1) * CR) * WbP
                    return lo, hi

                def issue_band(b, buf):
                    lo, hi = band_range(b)
                    for j in range(reph):
                        nc.sync.dma_start(buf[j * Kh:(j + 1) * Kh, lo:hi],
                                          H[0:Kh, lo + j:hi + j])

                # initial state (t=0): hrA holds zeros; xrA holds x/hseq slice 0
                nc.vector.memset(hrA[:, :], 0.0)
                build_xr(0, xrA)

                def step_body(t):
                    xr_cur, xr_nxt = (xrA, xrB) if t % 2 == 0 else (xrB, xrA)
                    hr_cur, hr_nxt = (hrA, hrB) if t % 2 == 0 else (hrB, hrA)
                    if t < T - 1:
                        build_xr(t + 1, xr_nxt)
                    xr3 = xr_cur[:, 0:FLATl].rearrange("p (h w) -> p h w", w=WbP)
                    hr3 = hr_cur[:, 0:FLATl].rearrange("p (h w) -> p h w", w=WbP)

                    for ci in range(NCHl):
                        r0 = PB + ci * CR
                        cs = slice(ci * CR, (ci + 1) * CR)
                        zts = []
                        for zh in range(nz):
                            zt = pp.tile([128, CR, Wb], F32, tag="z")
                            mms = [(wxt, xr3, s, bdy, bdx0)
                                   for (s, bdy, bdx0) in gx_list[zh]] + \
                                  [(wht, hr3, s, bdy, bdx0)
                                   for (s, bdy, bdx0) in gh_list[zh]]
                            for mi, (wt, rep3, s, bdy, bdx0) in enumerate(mms):
                                nc.tensor.matmul(
                                    zt[:, :, :],
                                    wt[:, s * 128:(s + 1) * 128],
                                    rep3[:, r0 + bdy:r0 + bdy + CR,
                                         PB + bdx0:PB + bdx0 + Wb],
                                    start=(mi == 0), stop=(mi == len(mms) - 1))
                            zts.append(zt)

                        # scalar engine drains PSUM -> bf16 staging
                        if nz == 2:
                            z1, z2 = zts
                            nc.scalar.activation(As[0:PA, cs, :], z1[:, :, :], AF.Relu,
                                                 bias=bia[:, 0:1], scale=0.2)
                            nc.scalar.activation(Gs[0:PG, cs, :], z2[0:PG, :, :], AF.Tanh,
                                                 bias=bib[0:PG, 0:1], scale=1.0)
                            nc.scalar.activation(Os[0:PG, cs, :], z2[PG:128, :, :], AF.Relu,
                                                 bias=bib[PG:128, 0:1], scale=0.2)
                        else:
                            z = zts[0]
                            nc.scalar.activation(As[0:PA, cs, :], z[0:PA, :, :], AF.Relu,
                                                 bias=bia[0:PA, 0:1], scale=0.2)
                            nc.scalar.activation(Gs[0:PG, cs, :], z[PA:PA + PG, :, :],
                                                 AF.Tanh, bias=bia[PA:PA + PG, 0:1],
                                                 scale=1.0)
                            nc.scalar.activation(Os[0:PG, cs, :], z[PA + PG:128, :, :],
                                                 AF.Relu, bias=bia[PA + PG:128, 0:1],
                                                 scale=0.2)

                        # group tail: big vector ops over NG chunks
                        if (ci + 1) % NG == 0:
                            g = ci // NG
                            c0 = g * NG
                            gs = slice(g * NGR, (g + 1) * NGR)
                            cg = C[NFp:2 * NFp, gs, :]
                            nc.vector.tensor_scalar_min(As[0:PA, gs, :], As[0:PA, gs, :],
                                                        1.0)
                            nc.vector.tensor_scalar_min(Os[0:PG, gs, :], Os[0:PG, gs, :],
                                                        1.0)
                            tg = tp.tile([128, NGR, Wb], F32, tag="t")
                            tgv = tg[NFp:2 * NFp, :, :]
                            nc.vector.tensor_tensor(tgv, As[0:NFp, gs, :],
                                                    Gs[0:NFp, gs, :], ALU.mult)
                            nc.vector.tensor_tensor(cg, As[NFp:2 * NFp, gs, :], cg,
                                                    ALU.mult)
                            nc.vector.tensor_tensor(cg, cg, tgv, ALU.add)
                            nc.scalar.activation(TCs[0:PG, gs, :], cg, AF.Tanh)
                            hw = H3[0:NFp, PB + g * NGR:PB + (g + 1) * NGR, PB:PB + Wb]
                            nc.vector.tensor_tensor(hw, Os[0:PG, gs, :],
                                                    TCs[0:PG, gs, :], ALU.mult)
                            if li == 3:
                                nc.vector.tensor_tensor(OS[:, gs, :], Os[0:PG, gs, :],
                                                        TCs[0:PG, gs, :], ALU.mult)
                            if t < T - 1:
                                for b in range(max(0, g * NG - 1),
                                               min((g + 1) * NG - 1, NCHl - 1)):
                                    issue_band(b, hr_nxt)
                    if t < T - 1:
                        issue_band(NCHl - 1, hr_nxt)

                    if li == 3:
                        nc.sync.dma_start(
                            out.ap()[bass.ds(t, 1), :, :],
                            OS[:, :, :].rearrange("p h w -> p (h w)"))
                    elif li == 0:
                        nc.sync.dma_start(hseqs[li].ap()[bass.ds(t + 1, 1), :, :],
                                          H[:, 0:FLAT2 + 8])
                    else:
                        # main part of hseq (no halo dependency)
                        nc.sync.dma_start(
                            hseqs[li].ap()[bass.ds(t + 1, 1), :, 0:HS - 8],
                            H[:, 0:HS - 8])

                    # --- halo exchange (L2..L4, skip last step of L4) ---
                    if li > 0 and not (li == 3 and t == T - 1):
                        TFp = 2 * Fp
                        nc.gpsimd.dma_start(SB[0:TFp, :, :],
                                            H3[TFp:NFp, PB + HBO - XR:PB + HBO, :])
                        nc.gpsimd.dma_start(SB[TFp:NFp, :, :],
                                            H3[0:TFp, PB + HBO - XR:PB + HBO, :])
                        sendt = dp.tile([NFp, XR * WbP], BF16, tag="send")
                        recvt = dp.tile([NFp, XR * WbP], BF16, tag="recv")
                        nc.gpsimd.dma_start(sendt[:, :],
                                            SB[0:NFp, :, :].rearrange("p h w -> p (h w)"))
                        nc.gpsimd.collective_compute(
                            "AllReduce", ALU.add, replica_groups=RG,
                            ins=[sendt[:, :]], outs=[recvt[:, :]])
                        nc.gpsimd.dma_start(Rt[0:NFp, :, :].rearrange("p h w -> p (h w)"),
                                            recvt[:, :])
                        for r in range(XR):
                            nc.vector.tensor_tensor(
                                H3[:, PB + HBO + r, :], Rt[0:NFp, XR - 1 - r, :],
                                SB[0:NFp, XR - 1 - r, :], ALU.subtract)
                        # halo band of hrep for t+1 + halo part of hseq store
                        if t < T - 1:
                            for j in range(reph):
                                nc.gpsimd.dma_start(
                                    hr_nxt[j * Kh:(j + 1) * Kh, HS - 8:FLAT2],
                                    H[0:Kh, HS - 8 + j:FLAT2 + j])
                        if li < 3:
                            nc.gpsimd.dma_start(
                                hseqs[li].ap()[bass.ds(t + 1, 1), :, HS - 8:FLAT2 + 8],
                                H[:, HS - 8:FLAT2 + 8])

                if static_unroll:
                    for t in range(T):
                        step_body(t)
                else:
                    with tc.For_i(0, T) as t:
                        step_body(t)
    nc.compile()
    return nc


# ------------------------------------------------------------------ runner --

_CACHED = {}
LAST_EXEC_NS = None


def _install_ntff_hook():
    """Provide the antenv.axon_hooks module this image lacks, backed by
    ctypes calls into libaxon_pjrt.so (same ABI trn_boot would use)."""
    import sys
    import types
    import ctypes
    import contextlib
    if 'antenv.axon_hooks' in sys.modules:
        return True
    try:
        lib = ctypes.CDLL('/opt/axon/libaxon_pjrt.so')
    except OSError:
        return False
    if not hasattr(lib, 'axon_start_nrt_profile'):
        return False
    lib.axon_start_nrt_profile.argtypes = [ctypes.POINTER(ctypes.c_int64),
                                           ctypes.c_size_t]
    lib.axon_start_nrt_profile.restype = ctypes.c_int64
    lib.axon_stop_nrt_profile.argtypes = [ctypes.c_char_p]
    lib.axon_stop_nrt_profile.restype = ctypes.c_int64

    @contextlib.contextmanager
    def _hook(output_dir, device_ids):
        import jax
        jax.devices()
        if device_ids:
            ids = (ctypes.c_int64 * len(device_ids))(*device_ids)
            rc = lib.axon_start_nrt_profile(ids, len(device_ids))
        else:
            rc = lib.axon_start_nrt_profile(None, 0)
        if rc != 0:
            raise RuntimeError(f'axon_start_nrt_profile rc={rc}')
        try:
            yield
        finally:
            n = lib.axon_stop_nrt_profile(str(output_dir).encode())
            print(f'ntff profile: {n} file(s) -> {output_dir}', flush=True)

    mod = types.ModuleType('antenv.axon_hooks')
    mod.get_axon_ntff_profile_hook = lambda: _hook
    mod.set_axon_ntff_profile_hook = lambda h: None
    sys.modules['antenv.axon_hooks'] = mod
    import concourse.bass_utils as bu
    bu.upload_artifacts = lambda tmpdir: 'local://' + tmpdir
    return True


def kernel(**inputs) -> np.ndarray:
    x = np.asarray(inputs['x'])
    B, Tt, Hf, Wf, _ = x.shape
    assert (Tt, Hf, Wf) == (T, 200, 200)
    if 'nc' not in _CACHED:
        _CACHED['nc'] = build_kernel(static_unroll=True)
    nc = _CACHED['nc']
    in_maps = [prep_core_inputs(inputs, b, half)
               for b in range(B) for half in range(2)]
    trace = bool(os.environ.get('KERNEL_TRACE')) and _install_ntff_hook()
    res = run_bass_kernel_spmd(nc, in_maps, core_ids=list(range(N_CORES)),
                               trace=trace,
                               tmpdir=os.environ.get('KERNEL_TRACE_DIR') or None)
    global LAST_EXEC_NS
    LAST_EXEC_NS = res.exec_time_ns
    outs = np.zeros((B, T, 2 * S * HBO, S * Wb, 5), np.float32)
    for b in range(B):
        for half in range(2):
            o = res.results[2 * b + half]['out']   # [T, 32, HBO*Wb]
            for t in range(T):
                img = un_s2d_np(o[t].reshape(32, HBO, Wb), 5, FPAD[3],
                                S * HBO, S * Wb)
                if half == 0:
                    outs[b, t, 0:100] = img
                else:
                    outs[b, t, 100:200] = img[::-1]
    if os.environ.get('KERNEL_TIME'):
        LAST_EXEC_NS = _timed_run(nc, in_maps,
                                  iters=int(os.environ.get('KERNEL_TIME_ITERS', '5')))
    return outs


def _timed_run(nc, in_maps, iters=5):
    """Wall-clock the NEFF execution via a non-donating jitted shard_map,
    device-resident inputs, min over iters. Returns ns."""
    import time
    import jax
    from jax.sharding import Mesh, PartitionSpec, NamedSharding
    from jax.experimental.shard_map import shard_map
    from concourse import bass2jax as b2j

    b2j.install_neuronx_cc_hook()
    partition_name = (nc.partition_id_tensor.name
                      if nc.partition_id_tensor else None)
    in_names, out_names, out_avals, zero_outs = [], [], [], []
    for alloc in nc.m.functions[0].allocations:
        if not isinstance(alloc, mybir.MemoryLocationSet):
            continue
        name = alloc.memorylocations[0].name
        if alloc.kind == "ExternalInput":
            if name != partition_name:
                in_names.append(name)
        elif alloc.kind == "ExternalOutput":
            shape = tuple(alloc.tensor_shape)
            npdt = mybir.dt.np(alloc.dtype)
            out_names.append(name)
            out_avals.append(jax.core.ShapedArray(shape, npdt))
            zero_outs.append(np.zeros(shape, npdt))
    n_params = len(in_names)
    in_names = in_names + out_names
    if partition_name is not None:
        in_names.append(partition_name)

    def _body(*args):
        operands = list(args)
        if partition_name is not None:
            operands.append(b2j.partition_id_tensor())
        outs = b2j._bass_exec_p.bind(
            *operands, out_avals=tuple(out_avals), in_names=tuple(in_names),
            out_names=tuple(out_names), lowering_input_output_aliases=(),
            sim_require_finite=True, sim_require_nnan=True, nc=nc)
        return tuple(outs)

    n = len(in_maps)
    devices = jax.devices()[:n]
    mesh = Mesh(np.asarray(devices), ("core",))
    sh = NamedSharding(mesh, PartitionSpec("core"))
    args = [jax.device_put(
                np.concatenate([np.asarray(in_maps[c][nm]) for c in range(n)], axis=0), sh)
            for nm in in_names[:n_params]]
    args += [jax.device_put(np.concatenate([z] * n, axis=0), sh) for z in zero_outs]
    f = jax.jit(shard_map(_body, mesh=mesh,
                          in_specs=(PartitionSpec("core"),) * (n_params + len(out_names)),
                          out_specs=(PartitionSpec("core"),) * len(out_names),
                          check_rep=False),
                keep_unused=True)
    ts = []
    for _ in range(iters + 1):
        t0 = time.perf_counter()
        o = f(*args)
        jax.block_until_ready(o)
        ts.append(time.perf_counter() - t0)
    best = min(ts[1:])
    print(f'timed_run wall times (s): {[f"{x:.4f}" for x in ts]}', flush=True)
    return int(best * 1e9)



# revision 20
# speedup vs baseline: 1.1717x; 1.1717x over previous
"""Trainium2 Bass kernel for stacked ConvLSTM2D (4 layers, Keras semantics).

Scheme: space-to-depth s=2 block layout; each conv is a sum of block-tap
matmuls with K padded to 128 via shift-baked replica buffers (all matmuls
K=128, M=128, N=CR*Wb, bf16 in / f32 PSUM).

8-core SPMD: batch b on core pair (2b, 2b+1), split over image rows.
Odd cores solve a vertically-FLIPPED half (flipped x + flipped weight slabs
prepped on host) so the program is rank-symmetric: every core owns local
block rows 0..49 (L1: 0..59), its local top is a true image boundary, and
its exchange edge is its local bottom.
 - L1 runs 60 rows with no exchange: validity of the extra 10 overlap rows
   decays 1 row/step; after 10 steps rows <=50 are still valid, exactly
   what L2 needs.
 - L2..L4 exchange a halo every step (pair AllReduce(add) in DRAM,
   halo = sum - own), written row-reversed into the frame bottom rows.

Interleaved wavefront phases (tensor-idle removal vs the layer-serial
baseline):
 - Phase A: wave w emits L1.step(w) then L2.step(w-1).  L2's x-conv
   replicas are READ DIRECTLY from L1's h-replica ring (identical K=32,
   reps=4 structure) -- no hseq1 DRAM roundtrip at all.
 - Phase B: wave w emits L3.step(w) then L4.step(w-1); L4's x-conv reads
   L3's h-replica ring (K=64, reps=2).  L2->L3 goes through DRAM hseq2.
 - Each layer's per-step serial tail (PSUM drain -> LSTM pointwise ->
   h-replica rebuild, plus the halo collective) is covered by the partner
   layer's matmuls that sit between consecutive steps in the engine queues.
 - t=0 skips all h-conv matmuls (h==0) and writes c = i*g directly, so
   replica buffers and C need no zero-init; tails use fused
   scalar_tensor_tensar ops ((x min 1) * y) so the hard-sigmoid clamp is
   free; nz=1 layers use gate order (i,f,o,g) so PSUM drains are 2
   activations; L1/L4 drain 2-chunk PSUM tiles to halve drain count.
Output is read straight from L4's H tile (bf16) and cast on host.
"""
import math
import os
from contextlib import ExitStack

import numpy as np
import ml_dtypes

import concourse.bacc as bacc
import concourse.bass as bass
import concourse.mybir as mybir
from concourse.tile import TileContext
from concourse.bass_utils import run_bass_kernel_spmd

BF16 = mybir.dt.bfloat16
F32 = mybir.dt.float32
AF = mybir.ActivationFunctionType
ALU = mybir.AluOpType

S = 2
PB = 4
# (cin_raw, F, k, tap_radius R)
LAYERS = [(1, 8, 3, 1), (8, 16, 5, 1), (16, 16, 9, 2), (16, 5, 12, 3)]
FPAD = [8, 16, 16, 8]
CINPAD = [1, 8, 16, 16]
# gate order in M (position -> original gate index; 0=i,1=f,2=g,3=o)
GORDER = [(0, 1, 3, 2), (0, 1, 2, 3), (0, 1, 2, 3), (0, 1, 3, 2)]
N_CORES = 8
RG = [[0, 1], [2, 3], [4, 5], [6, 7]]

Wb = 100
WbP = Wb + 2 * PB          # 108
HBO1 = 60                  # L1 owned block rows per core
HP1 = HBO1 + 2 * PB        # 68
FLAT1 = HP1 * WbP          # 7344
HBO = 50                   # L2..L4 owned block rows per core
HP2 = HBO + 2 * PB         # 58
FLAT2 = HP2 * WbP          # 6264
HS = (PB + HBO) * WbP      # 5832, halo start (frame row 54)
CR = 5
NG = 2                     # chunks per tail group
T = 10
XRL = [0, 2, 3, 3]


# ---------------------------------------------------------------- host prep --

def same_pad_lo(k):
    return (k - 1) // 2


def s2d_np(img):
    """[H, W, C] -> [4C, Hb, Wb], channel = siy*2C + c*2 + six (siy-major)."""
    H, W, C = img.shape
    Hb, Wbl = H // S, W // S
    t = img.reshape(Hb, S, Wbl, S, C)          # hb, siy, wb, six, c
    return t.transpose(1, 4, 3, 0, 2).reshape(4 * C, Hb, Wbl)


def un_s2d_np(blk, C, Cp, H, W):
    """blk [4Cp(siy,f,six), Hb, Wb] -> [H, W, C] (first C of Cp channels)."""
    Hb, Wbl = H // S, W // S
    b = blk.reshape(S, Cp, S, Hb, Wbl)[:, :C]  # siy, c, six, hb, wb
    return b.transpose(3, 0, 4, 2, 1).reshape(H, W, C)


def remap_kernel(Wk, cin_pad, F, Fp, order):
    k = Wk.shape[0]
    cin = Wk.shape[2]
    out = np.zeros((k, k, cin_pad, 4 * Fp), np.float32)
    for p, g in enumerate(order):
        out[:, :, :cin, p * Fp:p * Fp + F] = Wk[:, :, :, g * F:(g + 1) * F]
    return out


def block_weights(Wk, pt, pl, R):
    """Wk [k,k,cinp,4Fp] -> dict[(by,bx)] of [4cinp, 16Fp] with
    row = siy*2cinp + ci*2 + six, col = p*4Fp + soy*2Fp + f*2 + sox."""
    k, _, cinp, coutp = Wk.shape
    Fp = coutp // 4
    out = {}
    for by in range(-R, R + 1):
        for bx in range(-R, R + 1):
            M = np.zeros((4 * cinp, 4 * coutp), np.float32)
            Mv = M.reshape(4 * cinp, 4, S, Fp, S)
            for siy in range(S):
                for six in range(S):
                    for soy in range(S):
                        for sox in range(S):
                            dy = S * by + siy - soy + pt
                            dx = S * bx + six - sox + pl
                            if 0 <= dy < k and 0 <= dx < k:
                                rows = slice(siy * 2 * cinp + six,
                                             siy * 2 * cinp + 2 * cinp, 2)
                                Mv[rows, :, soy, :, sox] = \
                                    Wk[dy, dx].reshape(cinp, 4, Fp)
            out[(by, bx)] = M
    return out


def conv_groups(li, conv):
    """Returns (K, reps, groups) where groups = [(bdy, bdx0)]."""
    R = LAYERS[li][3]
    K = 4 * (CINPAD[li] if conv == 'x' else FPAD[li])
    reps = 128 // K
    ngroups_x = math.ceil((2 * R + 1) / reps)
    groups = [(bdy, -R + m * reps)
              for bdy in range(-R, R + 1) for m in range(ngroups_x)]
    return K, reps, groups


def build_slabs(Wblk, li, conv):
    """-> np [nz*G, 128, 128] f32 (caller casts to bf16). Order: [zh][group]."""
    R = LAYERS[li][3]
    K, reps, groups = conv_groups(li, conv)
    nz = 2 if FPAD[li] == 16 else 1
    slabs = []
    for zh in range(nz):
        for (bdy, bdx0) in groups:
            slab = np.zeros((128, 128), np.float32)
            for j in range(reps):
                bdx = bdx0 + j
                if bdx > R:
                    continue
                slab[j * K:(j + 1) * K, :] = Wblk[(bdy, bdx)][:, zh * 128:(zh + 1) * 128]
            slabs.append(slab)
    return np.stack(slabs)


def prep_core_inputs(inputs, b, half):
    """Build the input map for core 2b+half (half 1 = vertically flipped)."""
    m = {}
    for li, (cin, F, k, R) in enumerate(LAYERS):
        pl = same_pad_lo(k)
        pt = pl if half == 0 else k - 1 - pl
        Fp = FPAD[li]
        order = GORDER[li]
        Wx = np.asarray(inputs[f'Wx{li+1}'], np.float32)
        Wh = np.asarray(inputs[f'Wh{li+1}'], np.float32)
        if half == 1:
            Wx, Wh = Wx[::-1].copy(), Wh[::-1].copy()
        Wxb = block_weights(remap_kernel(Wx, CINPAD[li], F, Fp, order), pt, pl, R)
        Whb = block_weights(remap_kernel(Wh, Fp, F, Fp, order), pt, pl, R)
        braw = np.asarray(inputs[f'b{li+1}'], np.float32)
        bex = np.zeros(16 * Fp, np.float32)
        bexv = bex.reshape(4, S, Fp, S)
        for p, g in enumerate(order):
            for f in range(F):
                bexv[p, :, f, :] = braw[g * F + f]
        NFp = 4 * Fp
        if Fp == 16:
            # z1 = (i,f) hsig-pre; z2 = (g raw, o hsig-pre)
            b1 = 0.2 * bex[:2 * NFp] + 0.5
            b2 = np.concatenate([bex[2 * NFp:3 * NFp],
                                 0.2 * bex[3 * NFp:] + 0.5])
            m[f'bias{li+1}a'] = b1.reshape(128, 1)
            m[f'bias{li+1}b'] = b2.reshape(128, 1)
        else:
            # gate order (i,f,o,g): i,f,o hsig-pre then g raw
            b1 = np.concatenate([0.2 * bex[:3 * NFp] + 0.5,
                                 bex[3 * NFp:]])
            m[f'bias{li+1}a'] = b1.reshape(128, 1)
        if li == 0:
            # L1 x: single K=36 im2col slab (tap-major rows), 1 group
            slab = np.zeros((128, 128), np.float32)
            for t_i, (bdy, bdx) in enumerate(
                    (by, bx) for by in range(-R, R + 1) for bx in range(-R, R + 1)):
                slab[t_i * 4:(t_i + 1) * 4, :] = Wxb[(bdy, bdx)]
            m['wx1'] = slab[None].astype(ml_dtypes.bfloat16)
        else:
            m[f'wx{li+1}'] = build_slabs(Wxb, li, 'x').astype(ml_dtypes.bfloat16)
        m[f'wh{li+1}'] = build_slabs(Whb, li, 'h').astype(ml_dtypes.bfloat16)

    # L1 x im2col: local 120 pixel rows (60 block rows), frame 68 block rows
    x = np.asarray(inputs['x'], np.float32)[b]          # [T, 200, 200, 1]
    if half == 1:
        x = x[:, ::-1]
    xc = np.zeros((T, 36, FLAT1 + 8), np.float32)
    for t in range(T):
        xp = np.zeros((4, HP1, WbP), np.float32)
        xp[:, PB:PB + HBO1, PB:PB + Wb] = s2d_np(x[t, 0:2 * HBO1])
        flat = xp.reshape(4, FLAT1)
        for t_i, (bdy, bdx) in enumerate(
                (by, bx) for by in (-1, 0, 1) for bx in (-1, 0, 1)):
            sh = bdy * WbP + bdx
            for c in range(4):
                if sh >= 0:
                    xc[t, t_i * 4 + c, :FLAT1 - sh] = flat[c, sh:]
                else:
                    xc[t, t_i * 4 + c, -sh:FLAT1] = flat[c, :FLAT1 + sh]
    m['xcol'] = xc.astype(ml_dtypes.bfloat16)
    return m


# ------------------------------------------------------------- kernel build --

class LayerCtx:
    """Per-layer emission state + emitters."""

    def __init__(self, nc, tc, ls, li, wxt, wht, bia, bib):
        self.nc = nc
        self.li = li
        cin, F, k, R = LAYERS[li]
        self.R = R
        self.Fp = FPAD[li]
        self.NFp = 4 * self.Fp
        self.nz = 2 if self.Fp == 16 else 1
        self.Kx, self.repx, self.gx = conv_groups(li, 'x')
        self.Kh, self.reph, self.gh = conv_groups(li, 'h')
        self.FLAT = FLAT1 if li == 0 else FLAT2
        self.HBOl = HBO1 if li == 0 else HBO
        self.NCH = self.HBOl // CR
        self.XR = XRL[li]
        self.wxt, self.wht = wxt, wht
        self.bia, self.bib = bia, bib
        if li == 0:
            self.gx_list = [[(0, 0, 0)]]
        else:
            self.gx_list = [[(zh * len(self.gx) + i, bdy, bdx0)
                             for i, (bdy, bdx0) in enumerate(self.gx)]
                            for zh in range(self.nz)]
        self.gh_list = [[(zh * len(self.gh) + i, bdy, bdx0)
                         for i, (bdy, bdx0) in enumerate(self.gh)]
                        for zh in range(self.nz)]

    def band_range(self, b):
        lo = 0 if b == 0 else (PB + b * CR) * WbP
        if b == self.NCH - 1:
            hi = FLAT1 if self.li == 0 else HS - 8
        else:
            hi = (PB + (b + 1) * CR) * WbP
        return lo, hi

    def issue_band(self, b, buf):
        lo, hi = self.band_range(b)
        nc = self.nc
        # L2 bands on sync; L1/L3/L4 bands on gpsimd (keeps either queue
        # well under the wave period)
        eng = nc.sync if self.li == 1 else nc.gpsimd
        for j in range(self.reph):
            eng.dma_start(buf[j * self.Kh:(j + 1) * self.Kh, lo:hi],
                          self.H[0:self.Kh, lo + j:hi + j])


def build_kernel(dbg=False):
    nc = bacc.Bacc("TRN2", target_bir_lowering=False, debug=False,
                   num_devices=N_CORES)

    xcol_d = nc.dram_tensor('xcol', [T, 36, FLAT1 + 8], BF16, kind="ExternalInput")
    wts, biases = {}, {}
    for li in range(4):
        nz = 2 if FPAD[li] == 16 else 1
        Kx, repx, gxx = conv_groups(li, 'x')
        Kh, reph, ghh = conv_groups(li, 'h')
        Gx = 1 if li == 0 else nz * len(gxx)
        Gh = nz * len(ghh)
        wts[(li, 'x')] = nc.dram_tensor(f'wx{li+1}', [Gx, 128, 128], BF16, kind="ExternalInput")
        wts[(li, 'h')] = nc.dram_tensor(f'wh{li+1}', [Gh, 128, 128], BF16, kind="ExternalInput")
        biases[(li, 'a')] = nc.dram_tensor(f'bias{li+1}a', [128, 1], F32, kind="ExternalInput")
        if nz == 2:
            biases[(li, 'b')] = nc.dram_tensor(f'bias{li+1}b', [128, 1], F32, kind="ExternalInput")
    hseq2 = nc.dram_tensor('hseq2', [T + 1, 64, FLAT2 + 8], BF16,
                           kind="ExternalOutput" if dbg else "Internal")
    h1dbg = (nc.dram_tensor('h1dbg', [T, 32, FLAT1], BF16, kind="ExternalOutput")
             if dbg else None)
    h3dbg = (nc.dram_tensor('h3dbg', [T, 64, FLAT2], BF16, kind="ExternalOutput")
             if dbg else None)
    out = nc.dram_tensor('out', [T, 32, HBO * Wb], BF16, kind="ExternalOutput")

    with TileContext(nc) as tc, ExitStack() as top:

        def make_layer(ls, li, gp):
            """Allocate layer tiles + weight loads; return LayerCtx."""
            Gx = wts[(li, 'x')].shape[0]
            Gh = wts[(li, 'h')].shape[0]
            wxt = gp.tile([128, Gx * 128], BF16, tag=f"wx{li}", name=f"wxt{li}")
            wht = gp.tile([128, Gh * 128], BF16, tag=f"wh{li}", name=f"wht{li}")
            nc.sync.dma_start(wxt[:, :].rearrange("p (g c) -> p g c", c=128),
                              wts[(li, 'x')].ap().rearrange("g p c -> p g c"))
            nc.sync.dma_start(wht[:, :].rearrange("p (g c) -> p g c", c=128),
                              wts[(li, 'h')].ap().rearrange("g p c -> p g c"))
            bia = gp.tile([128, 1], F32, tag=f"ba{li}", name=f"bat{li}")
            nc.sync.dma_start(bia[:, :], biases[(li, 'a')].ap())
            bib = None
            if FPAD[li] == 16:
                bib = gp.tile([128, 1], F32, tag=f"bb{li}", name=f"bbt{li}")
                nc.sync.dma_start(bib[:, :], biases[(li, 'b')].ap())
            L = LayerCtx(nc, tc, ls, li, wxt, wht, bia, bib)
            lp = ls.enter_context(tc.tile_pool(name=f"l{li}", bufs=1))
            L.H = lp.tile([128, L.FLAT + 8], BF16, tag=f"H{li}", name=f"H{li}")
            L.H3 = L.H[:, 0:L.FLAT].rearrange("p (h w) -> p h w", w=WbP)
            L.C = lp.tile([128, L.HBOl, Wb], F32, tag=f"C{li}", name=f"C{li}")
            GR = NG * CR
            L.As = lp.tile([128, 2, GR, Wb], BF16, tag=f"As{li}", name=f"As{li}")
            L.Gs = lp.tile([128, 2, GR, Wb], BF16, tag=f"Gs{li}", name=f"Gs{li}")
            L.TCs = lp.tile([128, 2, GR, Wb], BF16, tag=f"TC{li}", name=f"TC{li}")
            L.tg = lp.tile([128, 2, GR, Wb], F32, tag=f"tg{li}", name=f"tg{li}")
            if li > 0:
                L.SB = lp.tile([128, L.XR, WbP], BF16, tag=f"SB{li}", name=f"SB{li}")
                L.Rt = lp.tile([128, L.XR, WbP], BF16, tag=f"Rt{li}", name=f"Rt{li}")
                L.dp = ls.enter_context(tc.tile_pool(name=f"xch{li}", bufs=2,
                                                     space="DRAM"))
            nc.vector.memset(L.H[:, :], 0.0)
            return L

        def emit_mms(L, t, ci, zt, zcols, x_src, h_src):
            """Matmuls for chunk ci into zt[:, zcols, :] (per zh list)."""
            r0 = PB + ci * CR
            for zh in range(L.nz):
                mms = [(L.wxt, x_src, s, bdy, bdx0)
                       for (s, bdy, bdx0) in L.gx_list[zh]]
                if t > 0:
                    mms += [(L.wht, h_src, s, bdy, bdx0)
                            for (s, bdy, bdx0) in L.gh_list[zh]]
                ztz = zt[zh]
                for mi, (wt, rep, s, bdy, bdx0) in enumerate(mms):
                    rep3 = rep[:, 0:L.FLAT].rearrange("p (h w) -> p h w", w=WbP)
                    nc.tensor.matmul(
                        ztz[:, zcols, :],
                        wt[:, s * 128:(s + 1) * 128],
                        rep3[:, r0 + bdy:r0 + bdy + CR,
                             PB + bdx0:PB + bdx0 + Wb],
                        start=(mi == 0), stop=(mi == len(mms) - 1))

        def emit_drain_nz1(L, ci, zt):
            """Per-chunk psum drain for nz=1 layers (gate order i,f,o,g)."""
            NFp = L.NFp
            P3 = 3 * NFp
            sl = (ci // NG) % 2
            rr = slice((ci % NG) * CR, (ci % NG) * CR + CR)
            nc.scalar.activation(L.As[0:P3, sl, rr, :], zt[0:P3, :, :],
                                 AF.Relu, bias=L.bia[0:P3, 0:1], scale=0.2)
            nc.scalar.activation(L.Gs[0:NFp, sl, rr, :], zt[P3:4 * NFp, :, :],
                                 AF.Tanh, bias=L.bia[P3:4 * NFp, 0:1], scale=1.0)

        def emit_drain_nz2(L, ci, zts):
            """Per-chunk psum drain for nz=2 layers (order i,f | g,o)."""
            sl = (ci // NG) % 2
            rr = slice((ci % NG) * CR, (ci % NG) * CR + CR)
            z1, z2 = zts
            nc.scalar.activation(L.As[0:128, sl, rr, :], z1[:, :, :],
                                 AF.Relu, bias=L.bia[:, 0:1], scale=0.2)
            nc.scalar.activation(L.Gs[0:64, sl, rr, :], z2[0:64, :, :],
                                 AF.Tanh, bias=L.bib[0:64, 0:1], scale=1.0)
            nc.scalar.activation(L.Gs[64:128, sl, rr, :], z2[64:128, :, :],
                                 AF.Relu, bias=L.bib[64:128, 0:1], scale=0.2)

        def emit_tail(L, g, t, out_t=None):
            """LSTM pointwise for group g (chunks 2g,2g+1).
            Partition placement keeps two-input ops on matching partitions
            (BIR samePartitionsAll): c lives at [NFp:2NFp] (aligned with the
            f gate in As), tanh(c) staging at the o gate's partitions."""
            NFp = L.NFp
            sl = g % 2
            GR = NG * CR
            gs = slice(g * GR, (g + 1) * GR)
            cC = L.C[NFp:2 * NFp, gs, :]
            if L.nz == 1:
                iA, fA, oA = (L.As[0:NFp, sl, :, :], L.As[NFp:2 * NFp, sl, :, :],
                              L.As[2 * NFp:3 * NFp, sl, :, :])
                gA = L.Gs[0:NFp, sl, :, :]
                ob = 2 * NFp
            else:
                iA, fA = L.As[0:64, sl, :, :], L.As[64:128, sl, :, :]
                gA, oA = L.Gs[0:64, sl, :, :], L.Gs[64:128, sl, :, :]
                ob = 64
            if t == 0:
                # c = hsig(i) * g~
                nc.vector.scalar_tensor_tensor(cC, iA, 1.0, gA, ALU.min, ALU.mult)
            else:
                tgv = L.tg[NFp:2 * NFp, sl, :, :]
                nc.vector.scalar_tensor_tensor(tgv, iA, 1.0, gA, ALU.min, ALU.mult)
                nc.vector.scalar_tensor_tensor(cC, fA, 1.0, cC, ALU.min, ALU.mult)
                nc.vector.tensor_tensor(cC, cC, tgv, ALU.add)
            tC = L.TCs[ob:ob + NFp, sl, :, :]
            nc.scalar.activation(tC, cC, AF.Tanh)
            hw = L.H3[0:NFp, PB + g * GR:PB + (g + 1) * GR, PB:PB + Wb]
            nc.vector.scalar_tensor_tensor(hw, oA, 1.0, tC, ALU.min, ALU.mult)

        def emit_halo(L, t, hr_nxt, rebuild):
            """Pair halo exchange on L.H; optionally rebuild hr halo band.
            The collective-blocked consumer DMAs go on sync so the gpsimd
            band-build queue never stalls on the CC latency."""
            NFp, XR, TFp = L.NFp, L.XR, 2 * L.Fp
            nc.gpsimd.dma_start(L.SB[0:TFp, :, :],
                                L.H3[TFp:NFp, PB + HBO - XR:PB + HBO, :])
            nc.gpsimd.dma_start(L.SB[TFp:NFp, :, :],
                                L.H3[0:TFp, PB + HBO - XR:PB + HBO, :])
            sendt = L.dp.tile([NFp, XR * WbP], BF16, tag="send", name=f"sd{L.li}")
            recvt = L.dp.tile([NFp, XR * WbP], BF16, tag="recv", name=f"rc{L.li}")
            nc.gpsimd.dma_start(sendt[:, :],
                                L.SB[0:NFp, :, :].rearrange("p h w -> p (h w)"))
            nc.gpsimd.collective_compute(
                "AllReduce", ALU.add, replica_groups=RG,
                ins=[sendt[:, :]], outs=[recvt[:, :]])
            nc.sync.dma_start(L.Rt[0:NFp, :, :].rearrange("p h w -> p (h w)"),
                              recvt[:, :])
            for r in range(XR):
                nc.vector.tensor_tensor(
                    L.H3[0:NFp, PB + HBO + r, :], L.Rt[0:NFp, XR - 1 - r, :],
                    L.SB[0:NFp, XR - 1 - r, :], ALU.subtract)
            if rebuild:
                for j in range(L.reph):
                    nc.sync.dma_start(
                        hr_nxt[j * L.Kh:(j + 1) * L.Kh, HS - 8:FLAT2],
                        L.H[0:L.Kh, HS - 8 + j:FLAT2 + j])

        # ---------------- generic step ----------------
        def step(L, t, x_src_fn, h_bufs, build_h, prefetch_fn=None,
                 post_tail_fn=None, post_step_fn=None):
            """Emit one layer-step.
            x_src_fn(t) -> replica buffer for x windows.
            h_bufs: list of own-h replica buffers (ring: [A,B]; single: [A]).
            build_h: emit band builds for t+1 (True except L4 t==T-1...)"""
            if prefetch_fn is not None:
                prefetch_fn(t)
            x_src = x_src_fn(t)
            h_src = h_bufs[t % len(h_bufs)]
            hr_nxt = h_bufs[(t + 1) % len(h_bufs)]
            for g in range(L.NCH // NG):
                for q in range(NG):
                    ci = g * NG + q
                    zts = [L.pp.tile([128, CR, Wb], F32, tag="z", name=f"z{L.li}")
                           for _ in range(L.nz)]
                    emit_mms(L, t, ci, zts, slice(0, CR), x_src, h_src)
                    if L.nz == 1:
                        emit_drain_nz1(L, ci, zts[0])
                    else:
                        emit_drain_nz2(L, ci, zts)
                emit_tail(L, g, t)
                if build_h:
                    for b in range(max(0, g * NG - 1),
                                   min((g + 1) * NG - 1, L.NCH - 1)):
                        L.issue_band(b, hr_nxt)
            if build_h:
                L.issue_band(L.NCH - 1, hr_nxt)
            if post_tail_fn is not None:
                post_tail_fn(t)
            if post_step_fn is not None:
                post_step_fn(t, hr_nxt)

        # ================= phase A: L1 + L2 =================
        with ExitStack() as pha:
            gp = pha.enter_context(tc.tile_pool(name="globA", bufs=1))
            L1 = make_layer(pha, 0, gp)
            L2 = make_layer(pha, 1, gp)
            L1.pp = pha.enter_context(tc.tile_pool(name="ps0", bufs=4, space="PSUM"))
            L2.pp = pha.enter_context(tc.tile_pool(name="ps1", bufs=4, space="PSUM"))
            # xcol ring + L1 h-replica ring (shared as L2's x source)
            xca = gp.tile([128, FLAT1], BF16, tag="xca")
            xcb = gp.tile([128, FLAT1], BF16, tag="xcb")
            hr1a = gp.tile([128, FLAT1], BF16, tag="hr1a")
            hr1b = gp.tile([128, FLAT1], BF16, tag="hr1b")
            hr2a = gp.tile([128, FLAT2], BF16, tag="hr2a")
            hr2b = gp.tile([128, FLAT2], BF16, tag="hr2b")
            xcr = [xca, xcb]
            hr1 = [hr1a, hr1b]
            # rows 36.. of xcol tiles are never written; zero once for the PE
            # (rows 0:36 get overwritten by the xcol loads afterwards)
            nc.vector.memset(xca[:, :], 0.0)
            nc.vector.memset(xcb[:, :], 0.0)

            def l1_prefetch(t):
                if t == 0:
                    nc.sync.dma_start(xcr[0][0:36, 0:FLAT1],
                                      xcol_d.ap()[bass.ds(0, 1), :, 0:FLAT1])
                if t < T - 1:
                    nc.sync.dma_start(xcr[(t + 1) % 2][0:36, 0:FLAT1],
                                      xcol_d.ap()[bass.ds(t + 1, 1), :, 0:FLAT1])

            def l2_store_main(t):
                nc.sync.dma_start(hseq2.ap()[bass.ds(t + 1, 1), :, 0:HS - 8],
                                  L2.H[0:64, 0:HS - 8])

            def l2_post(t, hr_nxt):
                emit_halo(L2, t, hr_nxt, rebuild=(t < T - 1))
                nc.gpsimd.dma_start(
                    hseq2.ap()[bass.ds(t + 1, 1), :, HS - 8:FLAT2 + 8],
                    L2.H[0:64, HS - 8:FLAT2 + 8])

            for w in range(T + 1):
                if w < T:
                    step(L1, w, lambda t: xcr[t % 2], hr1,
                         build_h=True, prefetch_fn=l1_prefetch)
                    if dbg:
                        nc.sync.dma_start(h1dbg.ap()[bass.ds(w, 1), :, :],
                                          L1.H[0:32, 0:FLAT1])
                if w >= 1:
                    step(L2, w - 1, lambda t: hr1[(t + 1) % 2], [hr2a, hr2b],
                         build_h=(w - 1 < T - 1),
                         post_tail_fn=l2_store_main, post_step_fn=l2_post)

        # ================= phase B: L3 + L4 =================
        with ExitStack() as phb:
            gp = phb.enter_context(tc.tile_pool(name="globB", bufs=1))
            L3 = make_layer(phb, 2, gp)
            L4 = make_layer(phb, 3, gp)
            L3.pp = phb.enter_context(tc.tile_pool(name="ps2", bufs=4, space="PSUM"))
            L4.pp = phb.enter_context(tc.tile_pool(name="ps3", bufs=4, space="PSUM"))
            xr3a = gp.tile([128, FLAT2], BF16, tag="xr3a")
            xr3b = gp.tile([128, FLAT2], BF16, tag="xr3b")
            hr3a = gp.tile([128, FLAT2], BF16, tag="hr3a")
            hr3b = gp.tile([128, FLAT2], BF16, tag="hr3b")
            hr4 = gp.tile([128, FLAT2], BF16, tag="hr4")
            xr3 = [xr3a, xr3b]
            hr3 = [hr3a, hr3b]

            def xr3_build(tt):
                for j in range(L3.repx):
                    nc.sync.dma_start(
                        xr3[tt % 2][j * L3.Kx:(j + 1) * L3.Kx, 0:FLAT2],
                        hseq2.ap()[bass.ds(tt + 1, 1), 0:L3.Kx, j:j + FLAT2])

            def l3_prefetch(t):
                if t == 0:
                    xr3_build(0)
                if t < T - 1:
                    xr3_build(t + 1)

            def l3_post(t, hr_nxt):
                emit_halo(L3, t, hr_nxt, rebuild=True)

            def l4_out(t):
                nc.sync.dma_start(
                    out.ap()[bass.ds(t, 1), :, :].rearrange(
                        "a p (h w) -> a p h w", w=Wb),
                    L4.H3[0:32, PB:PB + HBO, PB:PB + Wb])

            def l4_post(t, hr_nxt):
                if t < T - 1:
                    emit_halo(L4, t, hr_nxt, rebuild=True)

            for w in range(T + 1):
                if w < T:
                    step(L3, w, lambda t: xr3[t % 2], hr3,
                         build_h=True, prefetch_fn=l3_prefetch,
                         post_step_fn=l3_post)
                    if dbg:
                        nc.sync.dma_start(h3dbg.ap()[bass.ds(w, 1), :, :],
                                          L3.H[0:64, 0:FLAT2])
                if w >= 1:
                    step(L4, w - 1, lambda t: hr3[(t + 1) % 2], [hr4],
                         build_h=(w - 1 < T - 1),
                         post_tail_fn=l4_out, post_step_fn=l4_post)

    nc.compile()
    return nc


# ------------------------------------------------------------------ runner --

_CACHED = {}
LAST_EXEC_NS = None


def _install_ntff_hook():
    """Provide the antenv.axon_hooks module this image lacks, backed by
    ctypes calls into libaxon_pjrt.so (same ABI trn_boot would use)."""
    import sys
    import types
    import ctypes
    import contextlib
    if 'antenv.axon_hooks' in sys.modules:
        return True
    try:
        lib = ctypes.CDLL('/opt/axon/libaxon_pjrt.so')
    except OSError:
        return False
    if not hasattr(lib, 'axon_start_nrt_profile'):
        return False
    lib.axon_start_nrt_profile.argtypes = [ctypes.POINTER(ctypes.c_int64),
                                           ctypes.c_size_t]
    lib.axon_start_nrt_profile.restype = ctypes.c_int64
    lib.axon_stop_nrt_profile.argtypes = [ctypes.c_char_p]
    lib.axon_stop_nrt_profile.restype = ctypes.c_int64

    @contextlib.contextmanager
    def _hook(output_dir, device_ids):
        import jax
        jax.devices()
        if device_ids:
            ids = (ctypes.c_int64 * len(device_ids))(*device_ids)
            rc = lib.axon_start_nrt_profile(ids, len(device_ids))
        else:
            rc = lib.axon_start_nrt_profile(None, 0)
        if rc != 0:
            raise RuntimeError(f'axon_start_nrt_profile rc={rc}')
        try:
            yield
        finally:
            n = lib.axon_stop_nrt_profile(str(output_dir).encode())
            print(f'ntff profile: {n} file(s) -> {output_dir}', flush=True)

    mod = types.ModuleType('antenv.axon_hooks')
    mod.get_axon_ntff_profile_hook = lambda: _hook
    mod.set_axon_ntff_profile_hook = lambda h: None
    sys.modules['antenv.axon_hooks'] = mod
    import concourse.bass_utils as bu
    bu.upload_artifacts = lambda tmpdir: 'local://' + tmpdir
    return True


def kernel(**inputs) -> np.ndarray:
    x = np.asarray(inputs['x'])
    B, Tt, Hf, Wf, _ = x.shape
    assert (Tt, Hf, Wf) == (T, 200, 200)
    if 'nc' not in _CACHED:
        _CACHED['nc'] = build_kernel()
    nc = _CACHED['nc']
    in_maps = [prep_core_inputs(inputs, b, half)
               for b in range(B) for half in range(2)]
    trace = bool(os.environ.get('KERNEL_TRACE')) and _install_ntff_hook()
    res = run_bass_kernel_spmd(nc, in_maps, core_ids=list(range(N_CORES)),
                               trace=trace,
                               tmpdir=os.environ.get('KERNEL_TRACE_DIR') or None)
    global LAST_EXEC_NS
    LAST_EXEC_NS = res.exec_time_ns
    outs = np.zeros((B, T, 2 * S * HBO, S * Wb, 5), np.float32)
    for b in range(B):
        for half in range(2):
            o = np.asarray(res.results[2 * b + half]['out'], np.float32)
            for t in range(T):
                img = un_s2d_np(o[t].reshape(32, HBO, Wb), 5, FPAD[3],
                                S * HBO, S * Wb)
                if half == 0:
                    outs[b, t, 0:100] = img
                else:
                    outs[b, t, 100:200] = img[::-1]
    if os.environ.get('KERNEL_TIME'):
        LAST_EXEC_NS = _timed_run(nc, in_maps,
                                  iters=int(os.environ.get('KERNEL_TIME_ITERS', '5')))
    return outs


def _timed_run(nc, in_maps, iters=5):
    """Wall-clock the NEFF execution via a non-donating jitted shard_map,
    device-resident inputs, min over iters. Returns ns."""
    import time
    import jax
    from jax.sharding import Mesh, PartitionSpec, NamedSharding
    from jax.experimental.shard_map import shard_map
    from concourse import bass2jax as b2j

    b2j.install_neuronx_cc_hook()
    partition_name = (nc.partition_id_tensor.name
                      if nc.partition_id_tensor else None)
    in_names, out_names, out_avals, zero_outs = [], [], [], []
    for alloc in nc.m.functions[0].allocations:
        if not isinstance(alloc, mybir.MemoryLocationSet):
            continue
        name = alloc.memorylocations[0].name
        if alloc.kind == "ExternalInput":
            if name != partition_name:
                in_names.append(name)
        elif alloc.kind == "ExternalOutput":
            shape = tuple(alloc.tensor_shape)
            npdt = mybir.dt.np(alloc.dtype)
            out_names.append(name)
            out_avals.append(jax.core.ShapedArray(shape, npdt))
            zero_outs.append(np.zeros(shape, npdt))
    n_params = len(in_names)
    in_names = in_names + out_names
    if partition_name is not None:
        in_names.append(partition_name)

    def _body(*args):
        operands = list(args)
        if partition_name is not None:
            operands.append(b2j.partition_id_tensor())
        outs = b2j._bass_exec_p.bind(
            *operands, out_avals=tuple(out_avals), in_names=tuple(in_names),
            out_names=tuple(out_names), lowering_input_output_aliases=(),
            sim_require_finite=True, sim_require_nnan=True, nc=nc)
        return tuple(outs)

    n = len(in_maps)
    devices = jax.devices()[:n]
    mesh = Mesh(np.asarray(devices), ("core",))
    sh = NamedSharding(mesh, PartitionSpec("core"))
    args = [jax.device_put(
                np.concatenate([np.asarray(in_maps[c][nm]) for c in range(n)], axis=0), sh)
            for nm in in_names[:n_params]]
    args += [jax.device_put(np.concatenate([z] * n, axis=0), sh) for z in zero_outs]
    f = jax.jit(shard_map(_body, mesh=mesh,
                          in_specs=(PartitionSpec("core"),) * (n_params + len(out_names)),
                          out_specs=(PartitionSpec("core"),) * len(out_names),
                          check_rep=False),
                keep_unused=True)
    ts = []
    for _ in range(iters + 1):
        t0 = time.perf_counter()
        o = f(*args)
        jax.block_until_ready(o)
        ts.append(time.perf_counter() - t0)
    best = min(ts[1:])
    print(f'timed_run wall times (s): {[f"{x:.4f}" for x in ts]}', flush=True)
    return int(best * 1e9)
